# revision 23
# baseline (speedup 1.0000x reference)
"""Trainium2 Bass kernel for nn_ExtendedAnomalyNet (patch-CNN over 24x24 map).

Algorithm: multiPool decomposition of the 576 overlapping 33x33 patches:
conv1 is shared on the padded image; the two stride-2 maxpools become
parity-indexed pooled maps, so conv2/conv3 run once per parity combo
(~1.1 GMAC total vs 28.5 GMAC naive).

Sharding (8 cores): core c = (oy, ox, h): pool1 parity (oy, ox) in {0,1}^2
and spatial half h (output rows i<12 vs i>=12). Every stage after the
host-built conv1 im2col is core-local; each core produces 72 of the 576
output pixels (512 features each). No collectives; the host gathers.

Performance structure (vs 37us baseline):
- maxpool commutes with bias+LeakyReLU (both monotone) -> pool raw PSUM
  fp32 with single-input reduce-XY ops (2 for pool1, 4 for pool2), then
  activate 4x fewer elements with one fused bias+Lrelu ACT per stage.
- DMA priority: all queues share the 16 SDMA engines round-robin, so
  later transfers are gated on earlier ones (tiny reader ops on the
  issuing engine), keeping full HBM bandwidth on the critical tensor:
  r1 -> w2 -> w3/w45. r1 ships 75 real rows (no zero padding).
- PE warm-up: dummy matmuls on a zero tile keep TensorE busy from the
  prologue so the HAM clock-gate lifts 4/8 -> 8/8 before conv2 (the
  dominant matmul block) instead of after it.
- conv4 bias comes from a K=1 matmul (bias row x ones row) accumulated
  into PSUM, so both 128-channel halves share one zero-bias ACT.
- Tail: dense bias is added on the host; PSUM->SBUF fp16 conversion and
  the output DMA are split in two halves to overlap; output is fp16.
"""
import numpy as np

IMH = IMW = 24

_CACHE = {}


def _host_prep(x, c1w, c1b, c2w, c2b, c3w, c3b, c4w, c4b, c5w, c5b, dw, db):
    xp = np.pad(np.asarray(x, np.float32)[0], ((0, 0), (16, 16), (16, 16)))  # (3,56,56)
    sw = np.lib.stride_tricks.sliding_window_view(xp, (5, 5), axis=(1, 2))  # (3,52,52,5,5)
    w1 = np.asarray(c1w, np.float32).reshape(128, 75).T.astype(np.float16)  # (75,128)
    w2 = np.ascontiguousarray(
        np.asarray(c2w, np.float32).transpose(2, 3, 1, 0)  # (dy,dx,i,o)
    ).transpose(2, 0, 1, 3).reshape(128, 25 * 128).astype(np.float16)
    w3 = np.ascontiguousarray(
        np.asarray(c3w, np.float32).transpose(2, 3, 1, 0)
    ).transpose(2, 0, 1, 3).reshape(128, 25 * 128).astype(np.float16)
    w345 = np.zeros((128, 33, 128), np.float16)
    w345[:, 0:25, :] = w3.reshape(128, 25, 128)
    c4 = np.asarray(c4w, np.float32)[:, :, 0, 0]
    c5 = np.asarray(c5w, np.float32)[:, :, 0, 0]
    dwf = np.asarray(dw, np.float32)
    w345[:, 25, :] = c4[:128, :].T
    w345[:, 26, :] = c4[128:, :].T
    w345[:, 27, :] = c5[:, :128].T
    w345[:, 28, :] = c5[:, 128:].T
    for q in range(4):
        w345[:, 29 + q, :] = dwf[128 * q:128 * (q + 1), :].T
    w345 = w345.reshape(128, 4224)
    b4r = np.asarray(c4b, np.float32).reshape(1, 256).astype(np.float16)
    biases = np.zeros((128, 6), np.float32)
    biases[:, 0] = np.asarray(c1b, np.float32)
    biases[:, 1] = np.asarray(c2b, np.float32)
    biases[:, 2] = np.asarray(c3b, np.float32)
    biases[:, 5] = np.asarray(c5b, np.float32)
    _CACHE["db"] = np.asarray(db, np.float32)  # dense bias added on host
    in_maps = []
    for c in range(8):
        oy, ox, h = (c >> 2) & 1, (c >> 1) & 1, c & 1
        r0, c0 = oy + 12 * h, ox
        r1w = np.empty((75, 2028), np.float16)  # [w1 | conv1 im2col]
        r1w[:, 0:128] = w1
        r1w[:, 128:2028] = (
            sw[:, r0:r0 + 38, c0:c0 + 50, :, :]
            .transpose(0, 3, 4, 1, 2)
            .reshape(75, 38 * 50)
        ).astype(np.float16)
        in_maps.append({
            "r1w": r1w, "w2": w2, "w345": w345, "b4r": b4r, "biases": biases,
        })
    return in_maps


NSPAM1 = 24  # PE warm-up matmuls before conv1 (covers the DMA wait, cold rate)
NSPAM2 = 16  # warm-up matmuls between conv1 and conv2 (covers pool1, warm rate)
NSPAM3 = 46  # warm-up matmuls between conv2 and conv3 (covers pool2, warm rate)


def _build_nc():
    from contextlib import ExitStack

    import concourse.bass as bass
    import concourse.bacc as bacc
    import concourse.mybir as mybir
    import concourse.tile as tile

    dt = mybir.dt
    AF = mybir.ActivationFunctionType
    AX = mybir.AxisListType
    MAX = mybir.AluOpType.max

    nc = bacc.Bacc("TRN2", debug=False, num_devices=8)
    R1W = nc.dram_tensor("r1w", [75, 2028], dt.float16, kind="ExternalInput").ap()
    W2 = nc.dram_tensor("w2", [128, 3200], dt.float16, kind="ExternalInput").ap()
    W345 = nc.dram_tensor("w345", [128, 4224], dt.float16, kind="ExternalInput").ap()
    B4R = nc.dram_tensor("b4r", [1, 256], dt.float16, kind="ExternalInput").ap()
    BIAS = nc.dram_tensor("biases", [128, 6], dt.float32, kind="ExternalInput").ap()
    FEATS = nc.dram_tensor("feats", [128, 288], dt.float16, kind="ExternalOutput").ap()

    with tile.TileContext(nc) as tc, ExitStack() as ctx:
        const = ctx.enter_context(tc.tile_pool(name="const", bufs=1))
        work = ctx.enter_context(tc.tile_pool(name="work", bufs=1))
        ps = ctx.enter_context(tc.tile_pool(name="ps", bufs=1, space="PSUM"))

        r1w = const.tile([75, 2028], dt.float16)
        w1t = r1w[:, 0:128]
        r1t = r1w[:, 128:2028]
        w2t = const.tile([128, 25, 128], dt.float16)
        w345t = const.tile([128, 33, 128], dt.float16)
        b4rt = const.tile([1, 256], dt.float16)
        bt = const.tile([128, 6], dt.float32)

        # --- input DMAs ---
        # sync HWDGE ring (FIFO): conv1's inputs land first at full bandwidth.
        nc.sync.dma_start(out=r1w[:, 0:1128], in_=R1W[:, 0:1128])
        nc.sync.dma_start(out=r1w[:, 1128:2028], in_=R1W[:, 1128:2028])
        nc.gpsimd.dma_start(out=bt[:], in_=BIAS)
        nc.gpsimd.dma_start(out=b4rt[:], in_=B4R)

        # --- PE warm-up fodder + ACT table warm ---
        spam_sb = work.tile([128, 128], dt.float16)
        nc.vector.memset(spam_sb[:], 0.0)
        ones = work.tile([1, 128], dt.float16)
        nc.vector.memset(ones[:], 1.0)
        scratch = work.tile([1, 2], dt.float32)
        nc.scalar.activation(out=scratch[:], in_=spam_sb[0:1, 0:2], func=AF.Lrelu,
                             bias=0.0, scale=1.0, alpha=0.01)

        # w2's DMA rides the scalar HWDGE ring, delayed past r1's transfer
        # window by a short chain of dependent ACTs ending in a write to w2t
        # (WAR: the hoisted trigger cannot pass it). w345's DMA is gated on
        # w2's completion the same way, so each transfer gets full bandwidth.
        s2 = work.tile([1, 2], dt.float32)
        nc.scalar.activation(out=s2[:], in_=scratch[:], func=AF.Lrelu,
                             bias=0.0, scale=1.0, alpha=0.01)
        nc.scalar.activation(out=w2t[0:1, 0, 0:2], in_=s2[:], func=AF.Lrelu,
                             bias=0.0, scale=1.0, alpha=0.01)
        nc.scalar.dma_start(out=w2t[:], in_=W2.rearrange("p (t o) -> p t o", t=25))

        spam_ps = ps.tile([128, 128], dt.float32, tag="spam", bufs=1)
        for _ in range(NSPAM1):
            nc.tensor.matmul(spam_ps[:], spam_sb[:], spam_sb[:], start=True, stop=True)

        def lrelu_bias(dst, src, bias_col):
            nc.scalar.activation(
                out=dst, in_=src, func=AF.Lrelu,
                bias=bt[:, bias_col:bias_col + 1], scale=1.0, alpha=0.01,
            )

        # --- conv1: 4 chunks of 10/10/10/8 rows of the (38,50) grid ---
        rb = [0, 500, 1000, 1500, 1900]
        c1ps = []
        for n in range(4):
            sz = rb[n + 1] - rb[n]
            pc = ps.tile([128, 500], dt.float32, tag="c1", bufs=4)
            nc.tensor.matmul(pc[:, 0:sz], r1w[:, 0:128],
                             r1w[:, 128 + rb[n]:128 + rb[n + 1]],
                             start=True, stop=True)
            c1ps.append(pc)

        # --- pool1: per-chunk non-overlapping 2x2 max via reduce-XY on PSUM ---
        P1p = work.tile([128, 19, 25], dt.float16)
        for n in range(4):
            nr = (rb[n + 1] - rb[n]) // 50  # rows 10,10,10,8
            v = c1ps[n][:, 0:nr * 50].rearrange(
                "p (u a v b) -> p u v a b", a=2, v=25, b=2)
            nc.vector.tensor_reduce(out=P1p[:, 5 * n:5 * n + nr // 2], in_=v,
                                    axis=AX.XY, op=MAX)
        for _ in range(NSPAM2):
            nc.tensor.matmul(spam_ps[:], spam_sb[:], spam_sb[:], start=True, stop=True)
        P1 = work.tile([128, 19, 25], dt.float16)
        lrelu_bias(P1[:, 0:10], P1p[:, 0:10], 0)   # rows 0-9 (reduces 0,1)
        lrelu_bias(P1[:, 10:19], P1p[:, 10:19], 0)  # rows 10-18 (reduces 2,3)

        # w345's DMA: gated on w2's completion (read w2t tail, write w345t
        # corner) so it never steals bandwidth from w2.
        nc.scalar.activation(out=w345t[0:1, 0, 0:2], in_=w2t[0:1, 24, 0:2],
                             func=AF.Lrelu, bias=0.0, scale=1.0, alpha=0.01)
        nc.scalar.dma_start(out=w345t[:],
                            in_=W345.rearrange("p (t o) -> p t o", t=33))

        # --- conv2: 25 accumulating matmuls, N=15x21=315 ---
        p2 = ps.tile([128, 15, 21], dt.float32, tag="p2", bufs=1)
        for dy in range(5):
            for dx in range(5):
                t = dy * 5 + dx
                nc.tensor.matmul(p2[:], w2t[:, t, :], P1[:, dy:dy + 15, dx:dx + 21],
                                 start=(t == 0), stop=(t == 24))

        # --- pool2: per-parity-combo 2x2 max via reduce-XY on PSUM, one ACT ---
        for _ in range(NSPAM3):
            nc.tensor.matmul(spam_ps[:], spam_sb[:], spam_sb[:], start=True, stop=True)
        P2p = work.tile([128, 4, 7, 10], dt.float16)
        for py in range(2):
            for px in range(2):
                v = p2[:, py:py + 14, px:px + 20].rearrange(
                    "p (u a) (v b) -> p u v a b", a=2, b=2)
                nc.vector.tensor_reduce(out=P2p[:, 2 * py + px], in_=v,
                                        axis=AX.XY, op=MAX)
        P2 = work.tile([128, 4, 7, 10], dt.float16)
        lrelu_bias(P2[:, 0:2], P2p[:, 0:2], 1)  # combos (py=0, px=0/1)
        lrelu_bias(P2[:, 2:4], P2p[:, 2:4], 1)  # combos (py=1, px=0/1)

        # --- conv3: 25 accumulating matmuls, N=72 (combo, 3, 6) ---
        p3 = ps.tile([128, 288], dt.float32, tag="tail", bufs=2)
        for e in range(5):
            for f in range(5):
                t = e * 5 + f
                nc.tensor.matmul(p3[:, 0:72], w345t[:, t, :], P2[:, :, e:e + 3, f:f + 6],
                                 start=(t == 0), stop=(t == 24))
        h3 = work.tile([128, 72], dt.float16)
        lrelu_bias(h3[:], p3[:, 0:72], 2)

        # --- conv4: bias via K=1 matmul, both halves in one PSUM tile + one ACT ---
        p4 = ps.tile([128, 288], dt.float32, tag="tail", bufs=2)
        for half in range(2):
            nc.tensor.matmul(p4[:, 72 * half:72 * half + 72],
                             b4rt[0:1, 128 * half:128 * half + 128], ones[0:1, 0:72],
                             start=True, stop=False, skip_group_check=True)
        for half in range(2):
            nc.tensor.matmul(p4[:, 72 * half:72 * half + 72], w345t[:, 25 + half, :], h3[:],
                             start=False, stop=True, skip_group_check=True)
        h4 = work.tile([128, 2, 72], dt.float16)
        nc.scalar.activation(out=h4[:, 0], in_=p4[:, 0:72], func=AF.Lrelu,
                             bias=0.0, scale=1.0, alpha=0.01)
        nc.scalar.activation(out=h4[:, 1], in_=p4[:, 72:144], func=AF.Lrelu,
                             bias=0.0, scale=1.0, alpha=0.01)

        # --- conv5 (accumulate 2 K-halves) ---
        p5 = ps.tile([128, 288], dt.float32, tag="tail", bufs=2)
        nc.tensor.matmul(p5[:, 0:72], w345t[:, 27, :], h4[:, 0], start=True, stop=False)
        nc.tensor.matmul(p5[:, 0:72], w345t[:, 28, :], h4[:, 1], start=False, stop=True)
        h5 = work.tile([128, 72], dt.float16)
        lrelu_bias(h5[:], p5[:, 0:72], 5)

        # --- dense (4 quarters; fp16 convert + output DMA in 2 halves) ---
        pd = ps.tile([128, 288], dt.float32, tag="tail", bufs=2)
        outt = work.tile([128, 288], dt.float16)
        for q in range(4):
            nc.tensor.matmul(pd[:, 72 * q:72 * q + 72], w345t[:, 29 + q, :], h5[:],
                             start=True, stop=True)
            if q == 1:
                nc.vector.tensor_copy(out=outt[:, 0:144], in_=pd[:, 0:144])
                nc.sync.dma_start(out=FEATS[:, 0:144], in_=outt[:, 0:144])
        nc.vector.tensor_copy(out=outt[:, 144:288], in_=pd[:, 144:288])
        nc.sync.dma_start(out=FEATS[:, 144:288], in_=outt[:, 144:288])
    nc.compile()
    return nc


def _get_nc():
    if "nc" not in _CACHE:
        _CACHE["nc"] = _build_nc()
    return _CACHE["nc"]


def _run(in_maps, trace=False):
    from concourse.bass_utils import run_bass_kernel_spmd
    return run_bass_kernel_spmd(_get_nc(), in_maps, core_ids=list(range(8)),
                                trace=trace)


def _assemble(feats_list):
    db = _CACHE["db"]
    out = np.zeros((1, 512, IMH, IMW), np.float32)
    ii = np.arange(3)
    jj = np.arange(6)
    for c in range(8):
        oy, ox, h = (c >> 2) & 1, (c >> 1) & 1, c & 1
        f = np.asarray(feats_list[c], np.float32).reshape(128, 4, 72)
        f = f + db.reshape(4, 128).T[:, :, None]
        f = f.transpose(1, 0, 2).reshape(512, 4, 3, 6)
        for py in range(2):
            for px in range(2):
                i_idx = 4 * (3 * h + ii) + 2 * py + oy
                j_idx = 4 * jj + 2 * px + ox
                out[0, :, i_idx[:, None], j_idx[None, :]] = (
                    f[:, py * 2 + px].transpose(1, 2, 0)
                )
    return out


def kernel(**inputs):
    in_maps = _host_prep(**inputs)
    res = _run(in_maps)
    feats_list = [res.results[c]["feats"] for c in range(8)]
    return _assemble(feats_list)


# revision 24
# speedup vs baseline: 1.0945x; 1.0945x over previous
"""Trainium2 Bass kernel for nn_ExtendedAnomalyNet (patch-CNN over 24x24 map).

Algorithm: multiPool decomposition of the 576 overlapping 33x33 patches:
conv1 is shared on the padded image; the two stride-2 maxpools become
parity-indexed pooled maps, so conv2/conv3 run once per parity combo
(~1.1 GMAC total vs 28.5 GMAC naive).

Sharding (8 cores): core c = (oy, ox, h): pool1 parity (oy, ox) in {0,1}^2
and spatial half h (output rows i<12 vs i>=12). Every stage after the
host-built conv1 im2col is core-local; each core produces 72 of the 576
output pixels (512 features each). No collectives; the host gathers.

Performance structure (vs 37us baseline):
- maxpool commutes with bias+LeakyReLU (both monotone) -> pool raw PSUM
  fp32 with single-input reduce-XY ops (2 for pool1, 4 for pool2), then
  activate 4x fewer elements with one fused bias+Lrelu ACT per stage.
- DMA priority: all queues share the 16 SDMA engines round-robin, so
  later transfers are gated on earlier ones (tiny reader ops on the
  issuing engine), keeping full HBM bandwidth on the critical tensor:
  r1 -> w2 -> w3/w45. r1 ships 75 real rows (no zero padding).
- PE warm-up: dummy matmuls on a zero tile keep TensorE busy from the
  prologue so the HAM clock-gate lifts 4/8 -> 8/8 before conv2 (the
  dominant matmul block) instead of after it.
- conv4 bias comes from a K=1 matmul (bias row x ones row) accumulated
  into PSUM, so both 128-channel halves share one zero-bias ACT.
- Tail: dense bias is added on the host; PSUM->SBUF fp16 conversion and
  the output DMA are split in two halves to overlap; output is fp16.
"""
import numpy as np

IMH = IMW = 24

_CACHE = {}


def _host_prep(x, c1w, c1b, c2w, c2b, c3w, c3b, c4w, c4b, c5w, c5b, dw, db):
    xp = np.pad(np.asarray(x, np.float32)[0], ((0, 0), (16, 16), (16, 16)))  # (3,56,56)
    sw = np.lib.stride_tricks.sliding_window_view(xp, (5, 5), axis=(1, 2))  # (3,52,52,5,5)
    w1 = np.asarray(c1w, np.float32).reshape(128, 75).T.astype(np.float16)  # (75,128)
    w2 = np.ascontiguousarray(
        np.asarray(c2w, np.float32).transpose(2, 3, 1, 0)  # (dy,dx,i,o)
    ).transpose(2, 0, 1, 3).reshape(128, 25 * 128).astype(np.float16)
    w3 = np.ascontiguousarray(
        np.asarray(c3w, np.float32).transpose(2, 3, 1, 0)
    ).transpose(2, 0, 1, 3).reshape(128, 25 * 128).astype(np.float16)
    w345 = np.zeros((128, 33, 128), np.float16)
    w345[:, 0:25, :] = w3.reshape(128, 25, 128)
    c4 = np.asarray(c4w, np.float32)[:, :, 0, 0]
    c5 = np.asarray(c5w, np.float32)[:, :, 0, 0]
    dwf = np.asarray(dw, np.float32)
    w345[:, 25, :] = c4[:128, :].T
    w345[:, 26, :] = c4[128:, :].T
    w345[:, 27, :] = c5[:, :128].T
    w345[:, 28, :] = c5[:, 128:].T
    for q in range(4):
        w345[:, 29 + q, :] = dwf[128 * q:128 * (q + 1), :].T
    w345 = w345.reshape(128, 4224)
    b4r = np.asarray(c4b, np.float32).reshape(1, 256).astype(np.float16)
    biases = np.zeros((128, 6), np.float32)
    biases[:, 0] = np.asarray(c1b, np.float32)
    biases[:, 1] = np.asarray(c2b, np.float32)
    biases[:, 2] = np.asarray(c3b, np.float32)
    biases[:, 5] = np.asarray(c5b, np.float32)
    _CACHE["db"] = np.asarray(db, np.float32)  # dense bias added on host
    in_maps = []
    for c in range(8):
        oy, ox, h = (c >> 2) & 1, (c >> 1) & 1, c & 1
        r0, c0 = oy + 12 * h, ox
        r1w = np.empty((75, 2028), np.float16)  # [w1 | conv1 im2col]
        r1w[:, 0:128] = w1
        r1w[:, 128:2028] = (
            sw[:, r0:r0 + 38, c0:c0 + 50, :, :]
            .transpose(0, 3, 4, 1, 2)
            .reshape(75, 38 * 50)
        ).astype(np.float16)
        in_maps.append({
            "r1w": r1w, "w2": w2, "w345": w345, "b4r": b4r, "biases": biases,
        })
    return in_maps


NSPAM1 = 32  # PE warm-up matmuls before conv1 (covers the DMA wait, cold rate)
NSPAM2 = 7   # wide (N=512)  # warm-up matmuls between conv1 and conv2 (covers pool1, warm rate)
NSPAM3 = 46  # warm-up matmuls between conv2 and conv3 (covers pool2, warm rate)


def _build_nc():
    from contextlib import ExitStack

    import concourse.bass as bass
    import concourse.bacc as bacc
    import concourse.mybir as mybir
    import concourse.tile as tile

    dt = mybir.dt
    AF = mybir.ActivationFunctionType
    AX = mybir.AxisListType
    MAX = mybir.AluOpType.max

    nc = bacc.Bacc("TRN2", debug=False, num_devices=8)
    R1W = nc.dram_tensor("r1w", [75, 2028], dt.float16, kind="ExternalInput").ap()
    W2 = nc.dram_tensor("w2", [128, 3200], dt.float16, kind="ExternalInput").ap()
    W345 = nc.dram_tensor("w345", [128, 4224], dt.float16, kind="ExternalInput").ap()
    B4R = nc.dram_tensor("b4r", [1, 256], dt.float16, kind="ExternalInput").ap()
    BIAS = nc.dram_tensor("biases", [128, 6], dt.float32, kind="ExternalInput").ap()
    FEATS = nc.dram_tensor("feats", [128, 288], dt.float16, kind="ExternalOutput").ap()

    with tile.TileContext(nc) as tc, ExitStack() as ctx:
        const = ctx.enter_context(tc.tile_pool(name="const", bufs=1))
        work = ctx.enter_context(tc.tile_pool(name="work", bufs=1))
        ps = ctx.enter_context(tc.tile_pool(name="ps", bufs=1, space="PSUM"))

        r1w = const.tile([75, 2028], dt.float16)
        w1t = r1w[:, 0:128]
        r1t = r1w[:, 128:2028]
        w2t = const.tile([128, 25, 128], dt.float16)
        w345t = const.tile([128, 33, 128], dt.float16)
        b4rt = const.tile([1, 256], dt.float16)
        bt = const.tile([128, 6], dt.float32)

        # --- input DMAs ---
        # sync HWDGE ring (FIFO): conv1's inputs land first at full bandwidth.
        nc.sync.dma_start(out=r1w[:, 0:1128], in_=R1W[:, 0:1128])
        nc.sync.dma_start(out=r1w[:, 1128:2028], in_=R1W[:, 1128:2028])
        nc.gpsimd.dma_start(out=bt[:], in_=BIAS)
        nc.gpsimd.dma_start(out=b4rt[:], in_=B4R)

        # --- PE warm-up fodder + ACT table warm ---
        spam_sb = work.tile([128, 128], dt.float16)
        nc.vector.memset(spam_sb[:], 0.0)
        ones = work.tile([1, 128], dt.float16)
        nc.vector.memset(ones[:], 1.0)
        scratch = work.tile([1, 2], dt.float32)
        nc.scalar.activation(out=scratch[:], in_=spam_sb[0:1, 0:2], func=AF.Lrelu,
                             bias=0.0, scale=1.0, alpha=0.01)

        # w2's DMA rides the scalar HWDGE ring, delayed past r1's transfer
        # window by a short chain of dependent ACTs ending in a write to w2t
        # (WAR: the hoisted trigger cannot pass it). w345's DMA is gated on
        # w2's completion the same way, so each transfer gets full bandwidth.
        s2 = work.tile([1, 2], dt.float32)
        nc.scalar.activation(out=s2[:], in_=scratch[:], func=AF.Lrelu,
                             bias=0.0, scale=1.0, alpha=0.01)
        nc.scalar.activation(out=w2t[0:1, 0, 0:2], in_=s2[:], func=AF.Lrelu,
                             bias=0.0, scale=1.0, alpha=0.01)
        nc.scalar.dma_start(out=w2t[:], in_=W2.rearrange("p (t o) -> p t o", t=25))

        spam_ps = ps.tile([128, 512], dt.float32, tag="spam", bufs=1)
        spam_wide = work.tile([128, 512], dt.float16)
        nc.vector.memset(spam_wide[:], 0.0)
        for _ in range(NSPAM1):
            nc.tensor.matmul(spam_ps[:, 0:128], spam_sb[:], spam_sb[:],
                             start=True, stop=True)

        def lrelu_bias(dst, src, bias_col):
            nc.scalar.activation(
                out=dst, in_=src, func=AF.Lrelu,
                bias=bt[:, bias_col:bias_col + 1], scale=1.0, alpha=0.01,
            )

        # --- conv1: 4 chunks of 10/10/10/8 rows of the (38,50) grid ---
        rb = [0, 500, 1000, 1500, 1900]
        c1ps = []
        for n in range(4):
            sz = rb[n + 1] - rb[n]
            pc = ps.tile([128, 500], dt.float32, tag="c1", bufs=4)
            nc.tensor.matmul(pc[:, 0:sz], r1w[:, 0:128],
                             r1w[:, 128 + rb[n]:128 + rb[n + 1]],
                             start=True, stop=True)
            c1ps.append(pc)

        # --- pool1: per-chunk non-overlapping 2x2 max via reduce-XY on PSUM ---
        P1p = work.tile([128, 19, 25], dt.float16)
        for n in range(4):
            nr = (rb[n + 1] - rb[n]) // 50  # rows 10,10,10,8
            v = c1ps[n][:, 0:nr * 50].rearrange(
                "p (u a v b) -> p u v a b", a=2, v=25, b=2)
            nc.vector.tensor_reduce(out=P1p[:, 5 * n:5 * n + nr // 2], in_=v,
                                    axis=AX.XY, op=MAX)
        for _ in range(NSPAM2):
            nc.tensor.matmul(spam_ps[:], spam_sb[:], spam_wide[:],
                             start=True, stop=True)
        P1 = work.tile([128, 19, 25], dt.float16)
        lrelu_bias(P1[:, 0:10], P1p[:, 0:10], 0)   # rows 0-9 (reduces 0,1)
        lrelu_bias(P1[:, 10:19], P1p[:, 10:19], 0)  # rows 10-18 (reduces 2,3)

        # w345's DMA: gated on w2's completion (read w2t tail, write w345t
        # corner) so it never steals bandwidth from w2.
        nc.scalar.activation(out=w345t[0:1, 0, 0:2], in_=w2t[0:1, 24, 0:2],
                             func=AF.Lrelu, bias=0.0, scale=1.0, alpha=0.01)
        nc.scalar.dma_start(out=w345t[:],
                            in_=W345.rearrange("p (t o) -> p t o", t=33))

        # --- conv2: 25 accumulating matmuls, N=15x21=315 ---
        p2 = ps.tile([128, 15, 21], dt.float32, tag="p2", bufs=1)
        for dy in range(5):
            for dx in range(5):
                t = dy * 5 + dx
                nc.tensor.matmul(p2[:], w2t[:, t, :], P1[:, dy:dy + 15, dx:dx + 21],
                                 start=(t == 0), stop=(t == 24))

        # --- pool2: per-parity-combo 2x2 max via reduce-XY on PSUM, one ACT ---
        for _ in range(NSPAM3):
            nc.tensor.matmul(spam_ps[:, 0:128], spam_sb[:], spam_sb[:],
                             start=True, stop=True)
        P2p = work.tile([128, 4, 7, 10], dt.float16)
        for py in range(2):
            for px in range(2):
                v = p2[:, py:py + 14, px:px + 20].rearrange(
                    "p (u a) (v b) -> p u v a b", a=2, b=2)
                nc.vector.tensor_reduce(out=P2p[:, 2 * py + px], in_=v,
                                        axis=AX.XY, op=MAX)
        P2 = work.tile([128, 4, 7, 10], dt.float16)
        lrelu_bias(P2[:, 0:2], P2p[:, 0:2], 1)  # combos (py=0, px=0/1)
        lrelu_bias(P2[:, 2:4], P2p[:, 2:4], 1)  # combos (py=1, px=0/1)

        # --- conv3: 25 accumulating matmuls, N=72 (combo, 3, 6) ---
        p3 = ps.tile([128, 288], dt.float32, tag="tail", bufs=2)
        for e in range(5):
            for f in range(5):
                t = e * 5 + f
                nc.tensor.matmul(p3[:, 0:72], w345t[:, t, :], P2[:, :, e:e + 3, f:f + 6],
                                 start=(t == 0), stop=(t == 24))
        h3 = work.tile([128, 72], dt.float16)
        lrelu_bias(h3[:], p3[:, 0:72], 2)

        # --- conv4: bias via K=1 matmul, both halves in one PSUM tile + one ACT ---
        p4 = ps.tile([128, 288], dt.float32, tag="tail", bufs=2)
        for half in range(2):
            nc.tensor.matmul(p4[:, 72 * half:72 * half + 72],
                             b4rt[0:1, 128 * half:128 * half + 128], ones[0:1, 0:72],
                             start=True, stop=False, skip_group_check=True)
        for half in range(2):
            nc.tensor.matmul(p4[:, 72 * half:72 * half + 72], w345t[:, 25 + half, :], h3[:],
                             start=False, stop=True, skip_group_check=True)
        h4 = work.tile([128, 2, 72], dt.float16)
        nc.scalar.activation(out=h4[:], in_=p4[:, 0:144], func=AF.Lrelu,
                             bias=0.0, scale=1.0, alpha=0.01)

        # --- conv5 (accumulate 2 K-halves) ---
        p5 = ps.tile([128, 288], dt.float32, tag="tail", bufs=2)
        nc.tensor.matmul(p5[:, 0:72], w345t[:, 27, :], h4[:, 0], start=True, stop=False)
        nc.tensor.matmul(p5[:, 0:72], w345t[:, 28, :], h4[:, 1], start=False, stop=True)
        h5 = work.tile([128, 72], dt.float16)
        lrelu_bias(h5[:], p5[:, 0:72], 5)

        # --- dense (4 quarters; fp16 convert + output DMA in 2 halves) ---
        pd = ps.tile([128, 288], dt.float32, tag="tail", bufs=2)
        outt = work.tile([128, 288], dt.float16)
        for q in range(4):
            nc.tensor.matmul(pd[:, 72 * q:72 * q + 72], w345t[:, 29 + q, :], h5[:],
                             start=True, stop=True)
            if q == 1:
                nc.vector.tensor_copy(out=outt[:, 0:144], in_=pd[:, 0:144])
                nc.sync.dma_start(out=FEATS[:, 0:144], in_=outt[:, 0:144])
        nc.vector.tensor_copy(out=outt[:, 144:288], in_=pd[:, 144:288])
        nc.sync.dma_start(out=FEATS[:, 144:288], in_=outt[:, 144:288])
    nc.compile()
    return nc


def _get_nc():
    if "nc" not in _CACHE:
        _CACHE["nc"] = _build_nc()
    return _CACHE["nc"]


def _run(in_maps, trace=False):
    from concourse.bass_utils import run_bass_kernel_spmd
    return run_bass_kernel_spmd(_get_nc(), in_maps, core_ids=list(range(8)),
                                trace=trace)


def _assemble(feats_list):
    db = _CACHE["db"]
    out = np.zeros((1, 512, IMH, IMW), np.float32)
    ii = np.arange(3)
    jj = np.arange(6)
    for c in range(8):
        oy, ox, h = (c >> 2) & 1, (c >> 1) & 1, c & 1
        f = np.asarray(feats_list[c], np.float32).reshape(128, 4, 72)
        f = f + db.reshape(4, 128).T[:, :, None]
        f = f.transpose(1, 0, 2).reshape(512, 4, 3, 6)
        for py in range(2):
            for px in range(2):
                i_idx = 4 * (3 * h + ii) + 2 * py + oy
                j_idx = 4 * jj + 2 * px + ox
                out[0, :, i_idx[:, None], j_idx[None, :]] = (
                    f[:, py * 2 + px].transpose(1, 2, 0)
                )
    return out


def kernel(**inputs):
    in_maps = _host_prep(**inputs)
    res = _run(in_maps)
    feats_list = [res.results[c]["feats"] for c in range(8)]
    return _assemble(feats_list)
